# revision 32
# baseline (speedup 1.0000x reference)
"""Trainium2 Bass kernel for nn_EnhancedGCN42 (4-layer GCN + MLP classifier).

Strategy (8 NeuronCores, SPMD single NEFF):
  - Nodes dst-sharded: device d owns dst nodes [d*12500, (d+1)*12500).
  - A-hat = D^-1/2 (A+I) D^-1/2 factorized: tables store dis*h rows (bf16,
    256B rows); aggregation output scaled by dis_dst.
  - Table row space split into NR=4 regions sized [32,32,26,8] tiles/device;
    region q = [dev0 q | dev1 q | ... | dev7 q].  Each region has its OWN
    dram tensor per phase, AllGathered independently as soon as that
    region's tiles finish on every device -> next phase's range-q gathers
    depend only on region q (pipelined exchange, no dead windows).
  - AG triggers issued from the SP (sync) engine so they never head-of-line
    block the gpsimd gather queues.
  - Per layer: per-edge rows gathered via dma_gather (4 SWDGE queues,
    greedy-balanced), aggregated per 128-dst tile by matmul with an
    on-chip-built one-hot selection matrix (is_equal against iota, bf16).
  - Dense W / BN / ReLU fused per dst-tile in transposed layout; BN and
    classifier BN folded on host into per-feature scale/bias.

kernel(**inputs) -> [100000, 2] float32.
"""
import hashlib
import numpy as np
import ml_dtypes

import concourse.bacc as bacc
import concourse.bass as bass
import concourse.mybir as mybir
import concourse.tile as tile
from concourse.masks import make_identity
from concourse.bass_utils import run_bass_kernel_spmd

f32 = mybir.dt.float32
bf16 = mybir.dt.bfloat16
i16 = mybir.dt.int16
i32 = mybir.dt.int32
nbf16 = ml_dtypes.bfloat16

P = 128
NDEV = 8
NR = 4           # src index regions (int16 limit: region width <= 32768)
EPS = 1e-5
WTAB = 128       # table row = 128 cols bf16 = 256B
QT = [32, 32, 24, 10]  # tiles per region per device (sum = NT = 98)
BLK = 8


def _prep(x, edge_index, params, N):
    """Host preprocessing: graph partition + folded constants. Returns meta dict."""
    SHARD = N // NDEV
    NT = (SHARD + P - 1) // P
    TSHARD = NT * P
    TROWS = TSHARD * NDEV
    assert sum(QT) == NT
    TQ0 = np.cumsum([0] + QT)                      # tile starts per region
    RW = [NDEV * q * P for q in QT]                # region widths (rows)
    R0 = np.cumsum([0] + RW)                       # region row starts
    assert R0[NR] == TROWS and max(RW) <= 32768

    ei = edge_index.astype(np.int64)
    loop = np.arange(N, dtype=np.int64)
    src = np.concatenate([ei[0], loop])
    dst = np.concatenate([ei[1], loop])
    deg = np.bincount(dst, minlength=N).astype(np.float32)
    dis = (1.0 / np.sqrt(deg)).astype(np.float32)

    def padrow(n):
        s = n // SHARD
        i = n - s * SHARD
        t = i >> 7
        q = np.digitize(t, TQ0[1:NR])
        return R0[q] + (s * np.asarray(QT)[q] + (t - TQ0[q])) * P + (i & 127)

    psrc = padrow(src)

    # exclude appended self-loops (handled densely via identity matmul on
    # own-shard rows); keep coincidental src==dst edges from the random graph.
    nE = ei.shape[1]
    dst_e, psrc_e = dst[:nE], psrc[:nE]
    r_of = lambda pr: np.digitize(pr, R0[1:NR])

    counts = np.zeros((NDEV, NT, NR), dtype=np.int64)
    dev_edges = []
    for d in range(NDEV):
        m = (dst_e >= d * SHARD) & (dst_e < (d + 1) * SHARD)
        es_abs = psrc_e[m]
        el = dst_e[m] - d * SHARD
        t_id = el >> 7
        r_id = r_of(es_abs)
        order = np.lexsort((es_abs, r_id, t_id))  # (tile, region, src-ascending)
        es_abs, el, t_id, r_id = es_abs[order], el[order], t_id[order], r_id[order]
        np.add.at(counts[d], (t_id, r_id), 1)
        es_rel = es_abs - R0[r_id]                 # region-relative row index
        dev_edges.append((es_rel, el))

    grp_rows = ((counts.max(axis=0) + 15) // 16 * 16).astype(np.int64)  # [NT, NR]

    n_blk = (NT + BLK - 1) // BLK
    grp_off = np.zeros((NT, NR), dtype=np.int64)
    blk_off = np.zeros((n_blk, NR), dtype=np.int64)
    blk_rows = np.zeros((n_blk, NR), dtype=np.int64)
    acc = 0
    for b in range(n_blk):
        for r in range(NR):
            blk_off[b, r] = acc
            for t in range(b * BLK, min((b + 1) * BLK, NT)):
                grp_off[t, r] = acc
                acc += grp_rows[t, r]
            acc = (acc + P - 1) // P * P
            blk_rows[b, r] = acc - blk_off[b, r]
    TOT = acc

    # greedy queue balancing over (b, r) gathers by row count
    qload = [0, 0, 0, 0]
    qmap = {}
    for b in range(n_blk):
        for r in range(NR):
            qn = min(range(4), key=lambda i: qload[i])
            qmap[(b, r)] = qn
            qload[qn] += int(blk_rows[b, r])

    # chunk-use enumeration: per (t, r) the 128-row chunks its group overlaps.
    uses = [[[] for _ in range(NR)] for _ in range(NT)]
    n_uses = 0
    for b in range(n_blk):
        for r in range(NR):
            for t in range(b * BLK, min((b + 1) * BLK, NT)):
                g0, g1 = grp_off[t, r], grp_off[t, r] + grp_rows[t, r]
                c0, c1 = int(g0 // P), int((g1 + P - 1) // P)
                for ci in range(c0, c1):
                    uses[t][r].append((ci - int(blk_off[b, r]) // P, n_uses, ci))
                    n_uses += 1

    idx_w = np.zeros((NDEV, P, TOT // 16), dtype=np.int16)
    dstl_w = np.full((NDEV, P, n_uses), 255.0, dtype=np.float32)
    for d in range(NDEV):
        es, el = dev_edges[d]
        IDX = np.zeros(TOT, dtype=np.int16)
        DLOC = np.full(TOT, 255.0, dtype=np.float32)
        OWNER = np.full(TOT, -1, dtype=np.int64)
        pos = 0
        for t in range(NT):
            for r in range(NR):
                c = int(counts[d, t, r])
                o = int(grp_off[t, r])
                IDX[o:o + c] = es[pos:pos + c].astype(np.int16)
                DLOC[o:o + c] = (el[pos:pos + c] - t * P).astype(np.float32)
                OWNER[o:o + int(grp_rows[t, r])] = t
                pos += c
        idx_w[d] = np.tile(IDX.reshape(-1, 16).T, (8, 1))
        D = dstl_w[d]
        for t in range(NT):
            for r in range(NR):
                for (_lc, du, ci) in uses[t][r]:
                    rows = np.arange(ci * P, (ci + 1) * P)
                    v = np.where(OWNER[rows] == t, DLOC[rows], 255.0)
                    D[:, du] = v

    dis_pad = np.zeros((NDEV, TSHARD), dtype=np.float32)
    for s in range(NDEV):
        dis_pad[s, :SHARD] = dis[s * SHARD:(s + 1) * SHARD]
    dis_t = np.stack([dis_pad[d].reshape(NT, P).T for d in range(NDEV)])

    # Table 1 = dis * (x @ W1) (bf16), region-major row space; W1 folded on host
    xw = (x @ params["W1"].astype(np.float32)).astype(np.float32)
    v = (dis[:, None] * xw).astype(nbf16)
    xt = np.zeros((TROWS, WTAB), dtype=nbf16)
    xt[padrow(np.arange(N)), :v.shape[1]] = v
    # own-shard rows in local tile order per device
    xto = np.zeros((NDEV, TSHARD, WTAB), dtype=nbf16)
    for d in range(NDEV):
        xto[d, :SHARD, :v.shape[1]] = v[d * SHARD:(d + 1) * SHARD]
    # dis replicated across partitions, tile-order columns (for per-dst scale
    # of the transposed [feat, dst] aggregation output)
    disr = np.broadcast_to(dis_pad.reshape(NDEV, 1, TSHARD),
                           (NDEV, P, TSHARD)).astype(nbf16)

    def fold(g, be, rm, rv, b):
        k = (1.0 / np.sqrt(rv + EPS)).astype(np.float32)
        s = g * k
        t = (b - rm) * s + be
        return s.astype(np.float32), t.astype(np.float32)

    s1, t1 = fold(params["g1"], params["be1"], params["rm1"], params["rv1"], params["b1"])
    s2, t2 = fold(params["g2"], params["be2"], params["rm2"], params["rv2"], params["b2"])
    s3, t3 = fold(params["g3"], params["be3"], params["rm3"], params["rv3"], params["b3"])
    s4, t4 = fold(params["g4"], params["be4"], params["rm4"], params["rv4"], params["b4"])
    zk = (1.0 / np.sqrt(params["crv1"] + EPS)).astype(np.float32)
    cs1 = params["cg1"] * zk
    ct1 = -params["crm1"] * cs1 + params["cbe1"]
    zk = (1.0 / np.sqrt(params["crv2"] + EPS)).astype(np.float32)
    cs2 = params["cg2"] * zk
    ct2 = -params["crm2"] * cs2 + params["cbe2"]
    cW2p = (cs1[:, None] * params["cW2"]).astype(np.float32)
    cb2p = (ct1 @ params["cW2"] + params["cb2"]).astype(np.float32)
    cW3p = (cs2[:, None] * params["cW3"]).astype(np.float32)
    cb3p = (ct2 @ params["cW3"] + params["cb3"]).astype(np.float32)

    vecs = np.zeros((P, 13), dtype=np.float32)
    vecs[:, 0], vecs[:, 1] = s1, t1
    vecs[:, 2], vecs[:, 3] = s2[:128], t2[:128]
    vecs[:, 4], vecs[:, 5] = s2[128:], t2[128:]
    vecs[:, 6], vecs[:, 7] = s3, t3
    vecs[:64, 8], vecs[:64, 9] = s4, t4
    vecs[:64, 10] = params["cb1"]
    vecs[:32, 11] = cb2p
    vecs[:2, 12] = cb3p

    return dict(
        N=N, SHARD=SHARD, TSHARD=TSHARD, NT=NT, TROWS=TROWS,
        TQ0=[int(v) for v in TQ0], RW=[int(v) for v in RW],
        R0=[int(v) for v in R0],
        TOT=TOT, uses=uses, n_uses=n_uses, qmap=qmap,
        n_blk=n_blk, blk_off=blk_off, blk_rows=blk_rows,
        idx_w=idx_w, dstl_w=dstl_w, dis_t=dis_t, xt=xt, xto=xto, vecs=vecs,
        disr=disr, W2=params["W2"].astype(np.float32),
        W3=np.concatenate([params["W3"][:128], params["W3"][128:]], axis=1).astype(np.float32),
        W4=params["W4"].astype(np.float32),
        cW1=params["cW1"].astype(np.float32), cW2p=cW2p, cW3p=cW3p,
        d_in=x.shape[1],
    )


def _build(meta):
    """Build the Bass program (same for all cores)."""
    NT, TSHARD, TROWS = meta["NT"], meta["TSHARD"], meta["TROWS"]
    TQ0, RW, R0 = meta["TQ0"], meta["RW"], meta["R0"]
    TOT = meta["TOT"]
    uses, n_uses, qmap = meta["uses"], meta["n_uses"], meta["qmap"]
    n_blk, blk_off, blk_rows = meta["n_blk"], meta["blk_off"], meta["blk_rows"]

    def q_of(t):
        for q in range(NR):
            if t < TQ0[q + 1]:
                return q
        raise AssertionError

    nc = bacc.Bacc(None, target_bir_lowering=False, num_swdge_queues=4)
    t_xt = nc.dram_tensor("xt", [TROWS, WTAB], bf16, kind="ExternalInput")
    t_idx = nc.dram_tensor("idx", [P, TOT // 16], i16, kind="ExternalInput")
    t_dstl = nc.dram_tensor("dstl", [P, n_uses], bf16, kind="ExternalInput")
    t_xto = nc.dram_tensor("xt_own", [TSHARD, WTAB], bf16, kind="ExternalInput")
    t_dis = nc.dram_tensor("dis", [P, NT], f32, kind="ExternalInput")
    t_disr = nc.dram_tensor("disr", [P, TSHARD], bf16, kind="ExternalInput")
    t_vecs = nc.dram_tensor("vecs", [P, 13], f32, kind="ExternalInput")
    t_W2 = nc.dram_tensor("W2", [128, 256], f32, kind="ExternalInput")
    t_W3 = nc.dram_tensor("W3", [128, 256], f32, kind="ExternalInput")  # packed K-halves
    t_W4 = nc.dram_tensor("W4", [128, 64], f32, kind="ExternalInput")
    t_cW1 = nc.dram_tensor("cW1", [64, 64], f32, kind="ExternalInput")
    t_cW2 = nc.dram_tensor("cW2p", [64, 32], f32, kind="ExternalInput")
    t_cW3 = nc.dram_tensor("cW3p", [32, 2], f32, kind="ExternalInput")
    t_out = nc.dram_tensor("outT", [2, TSHARD], f32, kind="ExternalOutput")

    # per-phase per-region exchange buffers (local input, AllGathered output)
    cc_in = [[nc.dram_tensor(f"cc_in{k}_{q}", [QT[q] * P, WTAB], bf16)
              for q in range(NR)] for k in range(3)]
    tabs = [[nc.dram_tensor(f"tab{k}_{q}", [RW[q], WTAB], bf16, addr_space="Shared")
             for q in range(NR)] for k in range(3)]

    with tile.TileContext(nc) as tc:
        with (
            tc.tile_pool(name="const", bufs=1) as cpool,
            tc.tile_pool(name="gp", bufs=7) as gpool,
            tc.tile_pool(name="sp", bufs=12) as spool,
            tc.tile_pool(name="pagg", bufs=3, space="PSUM") as pagg,
            tc.tile_pool(name="paux", bufs=3, space="PSUM") as paux,
            tc.tile_pool(name="pacc", bufs=2, space="PSUM") as pacc,
            tc.tile_pool(name="ep", bufs=4) as ep,
        ):
            # ---- constants
            idx_sb = cpool.tile([P, TOT // 16], i16)
            nc.sync.dma_start(out=idx_sb[:], in_=t_idx[:])
            dstl_bf = cpool.tile([P, n_uses], bf16)
            nc.sync.dma_start(out=dstl_bf[:], in_=t_dstl[:])
            dstln_sb = cpool.tile([P, n_uses], f32)
            nc.vector.tensor_scalar_mul(dstln_sb[:], dstl_bf[:], -1.0)
            dis_sb = cpool.tile([P, NT], f32)
            nc.sync.dma_start(out=dis_sb[:], in_=t_dis[:])
            disr_sb = cpool.tile([P, TSHARD], bf16)
            nc.sync.dma_start(out=disr_sb[:], in_=t_disr[:])
            vecs_sb = cpool.tile([P, 13], f32)
            nc.sync.dma_start(out=vecs_sb[:], in_=t_vecs[:])
            W2_sb = cpool.tile([128, 256], f32)
            nc.sync.dma_start(out=W2_sb[:], in_=t_W2[:])
            W3_sb = cpool.tile([128, 256], f32)
            nc.sync.dma_start(out=W3_sb[:], in_=t_W3[:])
            W4_sb = cpool.tile([128, 64], f32)
            nc.sync.dma_start(out=W4_sb[:], in_=t_W4[:])
            cW1_sb = cpool.tile([64, 64], f32)
            nc.sync.dma_start(out=cW1_sb[:], in_=t_cW1[:])
            cW2_sb = cpool.tile([64, 32], f32)
            nc.sync.dma_start(out=cW2_sb[:], in_=t_cW2[:])
            cW3_sb = cpool.tile([32, 2], f32)
            nc.sync.dma_start(out=cW3_sb[:], in_=t_cW3[:])
            ident = cpool.tile([P, P], f32)
            make_identity(nc, ident[:])
            ident_bf = cpool.tile([P, P], bf16)
            nc.vector.tensor_copy(out=ident_bf[:], in_=ident[:])
            KMAX = max(len(uses[t][r]) for t in range(NT) for r in range(NR))
            iota_i = cpool.tile([P, KMAX, P], i32)
            nc.gpsimd.iota(iota_i[:], pattern=[[0, KMAX], [1, P]], base=0,
                           channel_multiplier=0)
            iota_bf = cpool.tile([P, KMAX, P], bf16)
            nc.vector.tensor_copy(out=iota_bf[:], in_=iota_i[:])

            AluEq = mybir.AluOpType.is_equal
            ACTF = mybir.ActivationFunctionType

            AluMul = mybir.AluOpType.mult

            def dis_scale(t, ps, w):
                """PSUM [w, dst] -> SBUF [w, dst] f32, scaled by dis[dst]."""
                a = ep.tile([w, P], f32, tag="adis")
                nc.vector.tensor_tensor(out=a[:], in0=ps[:],
                                        in1=disr_sb[:w, t * P:(t + 1) * P],
                                        op=AluMul)
                return a

            def ag_trigger(k, q):
                """AllGather region q of phase k (Pool-engine trigger)."""
                nc.gpsimd.collective_compute(
                    "AllGather", mybir.AluOpType.bypass,
                    replica_groups=[list(range(NDEV))],
                    ins=[cc_in[k][q][:]], outs=[tabs[k][q][:]],
                )

            def phase(table_of_r, own_of_t, w, epilogue, k_out):
                """Block-merged gathers + per-tile S-matmul aggregation.
                table_of_r(r) -> dram AP for region r (gather source).
                own_of_t(t) -> dram AP for own rows of tile t.
                k_out: phase index whose cc_in this phase's epilogue fills.
                AG triggers are placed in Pool program order two gather-blocks
                after the region's last tile, so their waits are met by the
                time they reach the queue head (no gather stall)."""
                due = {}   # block -> list of regions to AG after that block's gathers
                tail = []
                if k_out is not None:
                    for q in range(NR):
                        bb = (TQ0[q + 1] - 1) // BLK + 3
                        if bb < n_blk:
                            due.setdefault(bb, []).append(q)
                        elif (TQ0[q + 1] - 1) // BLK + 2 < n_blk:
                            due.setdefault(n_blk - 1, []).append(q)
                        else:
                            tail.append(q)
                for b in range(n_blk):
                    tiles = range(b * BLK, min((b + 1) * BLK, NT))
                    gt = {}
                    for r in range(NR):
                        rows = int(blk_rows[b, r])
                        if rows == 0:
                            continue
                        g = gpool.tile([P, rows // P, WTAB], bf16, tag="g")
                        off = int(blk_off[b, r])
                        nc.gpsimd.dma_gather(
                            out_ap=g[:],
                            in_ap=table_of_r(r),
                            idxs_ap=idx_sb[:, off // 16:(off + rows) // 16],
                            num_idxs=rows,
                            num_idxs_reg=rows,
                            elem_size=WTAB,
                            single_packet=False,
                            queue_num=qmap[(b, r)],
                        )
                        gt[r] = g
                    for q in due.get(b, []):
                        ag_trigger(k_out, q)
                    for t in tiles:
                        own = ep.tile([P, WTAB], bf16, tag="own")
                        nc.scalar.dma_start(out=own[:], in_=own_of_t(t))
                        spt = {}
                        for r in range(NR):
                            ul = uses[t][r]
                            if not ul:
                                continue
                            du0 = ul[0][1]
                            sP = spool.tile([P, len(ul), P], bf16, tag="s", bufs=12)
                            if (t * NR + r) % 4 != 3:
                                nc.vector.tensor_tensor(
                                    out=sP[:],
                                    in0=dstl_bf[:, du0:du0 + len(ul)].to_broadcast([P, len(ul), P]),
                                    in1=iota_bf[:, :len(ul), :],
                                    op=AluEq,
                                )
                            else:
                                # ACT path: s = Relu(1 - (iota - dstl)^2)
                                yq = spool.tile([P, len(ul), P], bf16, tag="yq", bufs=2)
                                for ui in range(len(ul)):
                                    nc.scalar.activation(
                                        yq[:, ui, :], iota_bf[:, ui, :], ACTF.Square,
                                        bias=dstln_sb[:, du0 + ui:du0 + ui + 1])
                                nc.scalar.activation(sP[:], yq[:], ACTF.Relu,
                                                     bias=1.0, scale=-1.0)
                            spt[r] = sP
                        # transposed aggregation: out [feat, dst]
                        nmm = 1 + sum(len(uses[t][r]) for r in range(NR))
                        ps = pagg.tile([w, P], f32, tag="pagg")
                        nc.tensor.matmul(ps[:], lhsT=own[:, :w], rhs=ident_bf[:],
                                         start=True, stop=(nmm == 1))
                        k = 1
                        for r in range(NR):
                            for ui, (lc, du, _ci) in enumerate(uses[t][r]):
                                nc.tensor.matmul(
                                    ps[:], lhsT=gt[r][:, lc, :w], rhs=spt[r][:, ui, :],
                                    start=False, stop=(k == nmm - 1),
                                )
                                k += 1
                        epilogue(t, ps)
                for q in tail:
                    ag_trigger(k_out, q)

            def cc_dst(k, t):
                q = q_of(t)
                return cc_in[k][q][(t - TQ0[q]) * P:(t - TQ0[q] + 1) * P, :]

            # ========== Phase 1: L1 (table1 = dis*(x@W1), host-folded) ==========
            def ep1(t, ps):
                a = dis_scale(t, ps, 128)          # (A' xW1)^T  [128, dst]
                hT = ep.tile([128, P], f32, tag="h1T")
                nc.scalar.activation(hT[:], a[:], ACTF.Relu,
                                     bias=vecs_sb[:, 1:2], scale=vecs_sb[:, 0:1])
                hp = paux.tile([P, 128], f32, tag="mm")
                nc.tensor.transpose(hp[:], hT[:], ident[:])
                hb = ep.tile([P, WTAB], bf16, tag="h1b")
                nc.scalar.activation(hb[:], hp[:], ACTF.Copy, scale=dis_sb[:, t:t + 1])
                nc.sync.dma_start(out=cc_dst(0, t), in_=hb[:])

            phase(lambda r: t_xt[R0[r]:R0[r] + RW[r], :],
                  lambda t: t_xto[t * P:(t + 1) * P, :], 128, ep1, 0)

            # ========== Phase 2: L2 + dense L3 (produce table3 = dis*(h2@W3)) ====
            def ep2(t, ps):
                a = dis_scale(t, ps, 128)          # (A' h1)^T  [128, dst]
                y3ps = pacc.tile([128, P], f32, tag="acc")
                for h in range(2):
                    hps = paux.tile([128, P], f32, tag="mm")
                    nc.tensor.matmul(hps[:], lhsT=W2_sb[:, h * 128:(h + 1) * 128],
                                     rhs=a[:], start=True, stop=True)
                    hT = ep.tile([128, P], f32, tag="h2T")
                    nc.scalar.activation(hT[:], hps[:], ACTF.Relu,
                                         bias=vecs_sb[:, 3 + 2 * h:4 + 2 * h],
                                         scale=vecs_sb[:, 2 + 2 * h:3 + 2 * h])
                    nc.tensor.matmul(y3ps[:], lhsT=W3_sb[:, h * 128:(h + 1) * 128],
                                     rhs=hT[:], start=(h == 0), stop=(h == 1))
                y3T = ep.tile([128, P], f32, tag="y3T")
                nc.vector.tensor_copy(out=y3T[:], in_=y3ps[:])
                y3p = paux.tile([P, 128], f32, tag="mm")
                nc.tensor.transpose(y3p[:], y3T[:], ident[:])
                y3b = ep.tile([P, WTAB], bf16, tag="y3b")
                nc.scalar.activation(y3b[:], y3p[:], ACTF.Copy, scale=dis_sb[:, t:t + 1])
                nc.sync.dma_start(out=cc_dst(1, t), in_=y3b[:])

            phase(lambda r: tabs[0][r][:], lambda t: cc_dst(0, t), 128, ep2, 1)

            # ========== Phase 3: L3 agg (table3 pre-BN) + dense L4 ==========
            def ep3(t, ps):
                z = dis_scale(t, ps, 128)          # (A'(h2 W3))^T  [128, dst]
                h3T = ep.tile([128, P], f32, tag="h3T")
                nc.scalar.activation(h3T[:], z[:], ACTF.Relu,
                                     bias=vecs_sb[:, 7:8], scale=vecs_sb[:, 6:7])
                y4ps = paux.tile([64, P], f32, tag="mm")
                nc.tensor.matmul(y4ps[:], lhsT=W4_sb[:], rhs=h3T[:], start=True, stop=True)
                y4T = ep.tile([64, P], f32, tag="y4T")
                nc.vector.tensor_copy(out=y4T[:], in_=y4ps[:])
                y4p = paux.tile([P, 64], f32, tag="mm")
                nc.tensor.transpose(y4p[:], y4T[:], ident[:64, :64])
                y4b = ep.tile([P, WTAB], bf16, tag="y4b")
                nc.vector.memset(y4b[:, 64:], 0)
                nc.scalar.activation(y4b[:, :64], y4p[:], ACTF.Copy,
                                     scale=dis_sb[:, t:t + 1])
                nc.sync.dma_start(out=cc_dst(2, t), in_=y4b[:])

            phase(lambda r: tabs[1][r][:], lambda t: cc_dst(1, t), 128, ep3, 2)

            # ========== Phase 4: L4 agg (table4 = dis*(h3@W4)) + classifier ======
            def ep4(t, ps):
                z = dis_scale(t, ps, 64)           # (A'(h3 W4))^T  [64, dst]
                h4T = ep.tile([64, P], f32, tag="h4T")
                nc.scalar.activation(h4T[:], z[:], ACTF.Relu,
                                     bias=vecs_sb[:64, 9:10], scale=vecs_sb[:64, 8:9])
                u1ps = paux.tile([64, P], f32, tag="mm")
                nc.tensor.matmul(u1ps[:], lhsT=cW1_sb[:], rhs=h4T[:], start=True, stop=True)
                u1T = ep.tile([64, P], f32, tag="u1T")
                nc.scalar.activation(u1T[:], u1ps[:], ACTF.Relu, bias=vecs_sb[:64, 10:11])
                u2ps = paux.tile([32, P], f32, tag="mm")
                nc.tensor.matmul(u2ps[:], lhsT=cW2_sb[:], rhs=u1T[:], start=True, stop=True)
                u2T = ep.tile([32, P], f32, tag="u2T")
                nc.scalar.activation(u2T[:], u2ps[:], ACTF.Relu, bias=vecs_sb[:32, 11:12])
                ops_ = paux.tile([2, P], f32, tag="mm")
                nc.tensor.matmul(ops_[:], lhsT=cW3_sb[:], rhs=u2T[:], start=True, stop=True)
                oT = ep.tile([2, P], f32, tag="oT")
                nc.scalar.activation(oT[:], ops_[:], ACTF.Identity, bias=vecs_sb[:2, 12:13])
                nc.sync.dma_start(out=t_out[:, t * P:(t + 1) * P], in_=oT[:])

            phase(lambda r: tabs[2][r][:], lambda t: cc_dst(2, t), 64, ep4, None)

    nc.finalize()
    return nc


_CACHE = {}


def kernel(**inputs):
    x = np.asarray(inputs["x"], dtype=np.float32)
    edge_index = np.asarray(inputs["edge_index"])
    N = x.shape[0]
    key = hashlib.sha256(edge_index.tobytes()).hexdigest()[:16] + f"_{N}_{x.shape[1]}"
    if key not in _CACHE:
        meta = _prep(x, edge_index, inputs, N)
        nc = _build(meta)
        _CACHE[key] = (meta, nc)
    else:
        meta, nc = _CACHE[key]
        meta = dict(meta)
        m2 = _prep(x, edge_index, inputs, N)
        meta.update({k: m2[k] for k in ("xt", "xto", "vecs", "disr", "W2", "W3",
                                        "W4", "cW1", "cW2p", "cW3p", "dis_t")})

    in_maps = []
    for d in range(NDEV):
        in_maps.append({
            "xt": meta["xt"],
            "xt_own": meta["xto"][d],
            "idx": meta["idx_w"][d],
            "dstl": meta["dstl_w"][d].astype(nbf16),
            "dis": meta["dis_t"][d],
            "vecs": meta["vecs"],
            "disr": meta["disr"][d],
            "W2": meta["W2"], "W3": meta["W3"], "W4": meta["W4"],
            "cW1": meta["cW1"], "cW2p": meta["cW2p"], "cW3p": meta["cW3p"],
        })
    SHARD = meta["SHARD"]
    out = np.empty((N, 2), dtype=np.float32)
    for _attempt in range(4):
        try:
            res = run_bass_kernel_spmd(nc, in_maps, core_ids=list(range(NDEV)), trace=False)
            for d in range(NDEV):
                out[d * SHARD:(d + 1) * SHARD] = res.results[d]["outT"][:, :SHARD].T
            if not np.isfinite(out).all():
                raise RuntimeError("non-finite output, retrying")
            break
        except Exception:
            if _attempt == 3:
                raise
    return out
